# revision 1
# baseline (speedup 1.0000x reference)
"""GRU cell (AnotherGRUCell) on 8 TRN2 NeuronCores.

Strategy: pure data-parallel over batch (8192 rows -> 1024 rows/core),
weights replicated. No collectives.

All on-chip compute is done in TRANSPOSED layout (units on the partition
axis, batch on the free axis):
  - matmul out[n, m] = sum_k W[k, n] * xT[k, m], with the weight tile as
    the stationary operand (lhsT) and xT/hT/rhT as the moving operand.
  - the r/u gate GEMMs x@Wi[:, :2u] + h@Wh[:, :2u] fuse into ONE 32-ktile
    PSUM accumulation over the concatenated operand [xT; hT].
  - the candidate GEMM x@Wi3 + (r*h)@Wh3 similarly accumulates over
    [xT; rhT]; rhT = sigmoid(gates) * hT is produced by ScalarE+VectorE
    already in the [k_part, m_free] layout the matmul needs -> zero
    on-chip transposes.
  - bias is per-partition in this layout, folded into the ScalarE
    activation (sigmoid/tanh) that reads PSUM directly.

The first gate pair is block-interleaved over the k loop so each
freshly-DMA'd x/h tile feeds 4 back-to-back matmuls (2 gate col-tiles x
2 batch chunks) into 4 PSUM banks, hiding the startup input-load
latency behind PE work; steady state interleaves the 2 batch chunks so
consecutive matmuls share the stationary weight tile.

Host side pre-transposes the x/h shards, packs weights into per-column-
tile slabs, casts to bf16, and transposes the [2048, 1024] per-core
output back to [1024, 2048].
"""

import numpy as np
import ml_dtypes

import concourse.bacc as bacc
import concourse.tile as tile
import concourse.mybir as mybir
from concourse.bass_utils import run_bass_kernel_spmd

N_CORES = 8
UNITS = 2048
IN_DIM = 2048
BATCH = 8192
B_LOC = BATCH // N_CORES  # 1024 batch rows per core

P = 128
KT_X = IN_DIM // P           # 16 k-tiles of x
KT_H = UNITS // P            # 16 k-tiles of h
KT = KT_X + KT_H             # 32 contraction k-tiles for [x; h]
NT_G = (2 * UNITS) // P      # 32 gate col-tiles (r: 0..15, u: 16..31)
NT_C = UNITS // P            # 16 candidate col-tiles
M_CHUNK = 512
MC = B_LOC // M_CHUNK        # 2 moving chunks per core

BF16 = mybir.dt.bfloat16
F32 = mybir.dt.float32
NP_BF16 = ml_dtypes.bfloat16

_CACHED_NC = None

# test.py sets TRACE=True to capture the NTFF profile (exec_time_ns +
# perfetto trace); the graded path leaves it off. LAST_RESULTS holds the
# BassKernelResults of the most recent run.
TRACE = False
LAST_RESULTS = None


def _build():
    nc = bacc.Bacc("TRN2", target_bir_lowering=False, debug=False)

    xT = nc.dram_tensor("xT", [KT_X, P, B_LOC], BF16, kind="ExternalInput")
    hT = nc.dram_tensor("hT", [KT_H, P, B_LOC], BF16, kind="ExternalInput")
    # Weights arrive packed in PAIRS of col-tiles: [pair, 128, 2*KT*128],
    # so each pair is one DMA -> one first-use wait on the PE queue.
    w_g = nc.dram_tensor("w_g", [NT_G // 2, P, 2 * KT * P], BF16,
                         kind="ExternalInput")
    w_c = nc.dram_tensor("w_c", [NT_C // 2, P, 2 * KT * P], BF16,
                         kind="ExternalInput")
    # biases transposed: one [128, n_tiles] tensor per gate set -> 1 DMA each
    b_g = nc.dram_tensor("b_g", [P, NT_G], F32, kind="ExternalInput")
    b_c = nc.dram_tensor("b_c", [P, NT_C], F32, kind="ExternalInput")
    out = nc.dram_tensor("out", [NT_C, P, B_LOC], F32, kind="ExternalOutput")

    SIG = mybir.ActivationFunctionType.Sigmoid
    TANH = mybir.ActivationFunctionType.Tanh

    with tile.TileContext(nc) as tc:
        with (
            tc.tile_pool(name="resident", bufs=1) as res,
            tc.tile_pool(name="wslab", bufs=4) as wp,
            tc.tile_pool(name="psum", bufs=8, space="PSUM") as pp,
            tc.tile_pool(name="stage", bufs=2) as sp,
            tc.tile_pool(name="bias", bufs=1) as bp,
        ):
            x_tiles = [
                res.tile([P, B_LOC], BF16, tag=f"x{j}", name=f"x{j}")
                for j in range(KT_X)
            ]
            h_tiles = [
                res.tile([P, B_LOC], BF16, tag=f"h{j}", name=f"h{j}")
                for j in range(KT_H)
            ]
            rh_tiles = [
                res.tile([P, B_LOC], BF16, tag=f"rh{j}", name=f"rh{j}")
                for j in range(KT_H)
            ]
            u_tiles = [
                res.tile([P, B_LOC], BF16, tag=f"u{j}", name=f"u{j}")
                for j in range(NT_C)
            ]

            # PE warm-up: the HAM clock gate holds the PE at 1.2 GHz until
            # it has been busy ~3.4us. The first real matmul can't start
            # before ~11us (preamble + first input DMA), so 8 dummy
            # matmuls (~3.4us at cold rate) fill the 7..10.5us window and
            # un-throttle the PE right before real data lands. Sized so
            # they END before the first operands arrive.
            warm_src = sp.tile(
                [P, M_CHUNK], BF16, tag="warm", name="warm_src", bufs=1
            )
            nc.gpsimd.memset(warm_src[:], 0.0)
            warm_ps = pp.tile([P, M_CHUNK], F32, tag="psum", name="warm_ps")
            for w in range(8):
                nc.tensor.matmul(
                    warm_ps[:],
                    warm_src[:, :P],
                    warm_src[:],
                    start=(w == 0),
                    stop=(w == 7),
                )

            # Startup DMAs in exact consumption order of the first gate
            # pair, interleaved across both HWDGE rings. The rings drain
            # roughly FIFO at HBM rate, so block jb's operands (weight
            # chunk jb/8 of both slabs + src tiles jb..jb+7) are emitted
            # right before the block needs them.
            # Graduated chunk sizes: a tiny first chunk so the very first
            # matmul's dependencies are only ~0.6MB of DMA.
            CHUNKS = [2, 6, 8, 8, 8]
            CB = [0, 2, 8, 16, 24, 32]  # chunk k-tile boundaries
            ws_first = [[None] * len(CHUNKS) for _ in range(2)]  # [t][chunk]
            src_dma = {}  # j -> (engine, dst tile, src ap)
            for j in range(KT_X):
                eng = nc.sync if j % 2 == 0 else nc.scalar
                src_dma[j] = (eng, x_tiles[j], xT[j, :, :])
            for j in range(KT_H):
                eng = nc.scalar if j % 2 == 0 else nc.sync
                src_dma[KT_X + j] = (eng, h_tiles[j], hT[j, :, :])
            for c, cw in enumerate(CHUNKS):
                for t in range(2):
                    w = wp.tile(
                        [P, cw * P], BF16, tag=f"wg{t}_{c}", name=f"wg{t}_{c}",
                        bufs=1,
                    )
                    (nc.sync if t == 0 else nc.scalar).dma_start(
                        w[:],
                        w_g[0, :, (t * KT + CB[c]) * P:(t * KT + CB[c + 1]) * P],
                    )
                    ws_first[t][c] = w
                for j in range(CB[c], CB[c + 1]):
                    eng, dst, src = src_dma[j]
                    if c == 0:
                        # First block: land x0/x1 via the SWDGE queue, in
                        # parallel with the weight chunks on the two HWDGE
                        # rings, so the very first matmuls unblock sooner.
                        eng = nc.gpsimd
                    eng.dma_start(dst[:], src)

            bg_all = bp.tile([P, NT_G], F32, tag="bg", name="bg_all")
            nc.sync.dma_start(bg_all[:], b_g[:, :])
            bc_all = bp.tile([P, NT_C], F32, tag="bc", name="bc_all")
            nc.scalar.dma_start(bc_all[:], b_c[:, :])

            gate_src = x_tiles + h_tiles
            cand_src = x_tiles + rh_tiles

            def act_sig(t, m, ps):
                """sigmoid(psum + b) -> rh (r gates, premultiplied by h) or u."""
                ms = slice(m * M_CHUNK, (m + 1) * M_CHUNK)
                if t < NT_C:
                    rt = sp.tile([P, M_CHUNK], BF16, tag="rtmp", name=f"r{t}_{m}")
                    nc.scalar.activation(rt[:], ps[:], SIG, bias=bg_all[:, t:t + 1])
                    nc.vector.tensor_mul(rh_tiles[t][:, ms], rt[:], h_tiles[t][:, ms])
                else:
                    nc.scalar.activation(
                        u_tiles[t - NT_C][:, ms], ps[:], SIG,
                        bias=bg_all[:, t:t + 1],
                    )

            # Phase 1: gates; t 0..15 -> r, 16..31 -> u.
            #
            # The FIRST pair of gate tiles is block-interleaved (4 psum
            # groups, one block per weight chunk) so the PE has 4 matmuls
            # to run per freshly-arrived x/h tile during the startup
            # loads instead of stalling on the whole operand set.
            t0_groups = [(0, 0), (0, 1), (1, 0), (1, 1)]
            pss0 = [
                pp.tile([P, M_CHUNK], F32, tag="psum", name=f"psg0_{i}")
                for i in range(4)
            ]
            for c in range(len(CHUNKS)):
                for i, (t, m) in enumerate(t0_groups):
                    ms = slice(m * M_CHUNK, (m + 1) * M_CHUNK)
                    for j in range(CB[c], CB[c + 1]):
                        wch = ws_first[t][c]
                        jj = j - CB[c]
                        nc.tensor.matmul(
                            pss0[i][:],
                            wch[:, jj * P:(jj + 1) * P],
                            gate_src[j][:, ms],
                            start=(j == 0),
                            stop=(j == KT - 1),
                        )
            for i, (t, m) in enumerate(t0_groups):
                act_sig(t, m, pss0[i])

            # Weight slabs are loaded in PAIRS (two gate col-tiles per DMA):
            # the PE pays one skipped 216ns beat at each new weight tile's
            # first-use semaphore wait, so halving the tile count halves
            # that cost. Within each t the k loop is m-interleaved so
            # consecutive matmuls share the stationary weight tile.
            def slab_pair(w_dram, tp, name):
                ws = wp.tile([P, 2 * KT * P], BF16, tag="wslab", name=name, bufs=2)
                (nc.sync if (tp // 2) % 2 == 0 else nc.scalar).dma_start(
                    ws[:], w_dram[tp // 2, :, :]
                )
                return ws

            def gemm_group(ws, ti, src_tiles, t, act_fn):
                """One m-interleaved accumulation pair for gate col-tile t,
                using half `ti` of the pair slab `ws`."""
                psl = [
                    pp.tile([P, M_CHUNK], F32, tag="psum", name=f"ps{t}_{m}")
                    for m in range(MC)
                ]
                for j in range(KT):
                    off = (ti * KT + j) * P
                    for m in range(MC):
                        ms = slice(m * M_CHUNK, (m + 1) * M_CHUNK)
                        nc.tensor.matmul(
                            psl[m][:],
                            ws[:, off:off + P],
                            src_tiles[j][:, ms],
                            start=(j == 0),
                            stop=(j == KT - 1),
                        )
                for m in range(MC):
                    act_fn(t, m, psl[m])

            for tp in range(2, NT_G, 2):
                ws = slab_pair(w_g, tp, f"wg{tp}")
                for ti, t in enumerate((tp, tp + 1)):
                    gemm_group(ws, ti, gate_src, t, act_sig)

            # Phase 2: candidate GEMM + tanh + output combine
            # h_t = u * (h - cand) + cand
            def cand_epilogue(t, m, mw, ps):
                ms = slice(m * mw, (m + 1) * mw)
                cand = sp.tile([P, mw], F32, tag="cand", name=f"c{t}_{m}")
                nc.scalar.activation(cand[:], ps[:], TANH, bias=bc_all[:, t:t + 1])
                d = sp.tile([P, mw], F32, tag="d", name=f"d{t}_{m}")
                nc.vector.tensor_sub(d[:], h_tiles[t][:, ms], cand[:])
                d2 = sp.tile([P, mw], F32, tag="d2", name=f"d2{t}_{m}")
                nc.vector.tensor_mul(d2[:], u_tiles[t][:, ms], d[:])
                ht = sp.tile([P, mw], F32, tag="ht", name=f"ht{t}_{m}")
                nc.vector.tensor_add(ht[:], d2[:], cand[:])
                nc.sync.dma_start(out[t, :, ms], ht[:])

            def cand_group(ws, ti, t):
                gemm_group(
                    ws, ti, cand_src, t,
                    lambda t, m, ps: cand_epilogue(t, m, M_CHUNK, ps),
                )

            def cand_group_tapered(ws, ti, t):
                # Last tile: narrow sequential chunks so epilogues stagger
                # and the post-final-matmul tail stays short.
                mw = M_CHUNK // 2
                for m in range(B_LOC // mw):
                    ms = slice(m * mw, (m + 1) * mw)
                    ps = pp.tile([P, mw], F32, tag="psum", name=f"psc{t}_{m}")
                    for j in range(KT):
                        off = (ti * KT + j) * P
                        nc.tensor.matmul(
                            ps[:],
                            ws[:, off:off + P],
                            cand_src[j][:, ms],
                            start=(j == 0),
                            stop=(j == KT - 1),
                        )
                    cand_epilogue(t, m, mw, ps)

            for tp in range(0, NT_C, 2):
                ws = slab_pair(w_c, tp, f"wc{tp}")
                for ti, t in enumerate((tp, tp + 1)):
                    if t == NT_C - 1:
                        cand_group_tapered(ws, ti, t)
                    else:
                        cand_group(ws, ti, t)

    nc.compile()
    return nc


def _get_nc():
    global _CACHED_NC
    if _CACHED_NC is None:
        _CACHED_NC = _build()
    return _CACHED_NC


def _pack_w(w):
    """[K, N] fp32 -> [N/128, 128, K] bf16 slab layout:
    slab[t, p, j*128 + c] = w[j*128 + p, t*128 + c]"""
    K, N = w.shape
    a = w.reshape(K // P, P, N // P, P).transpose(2, 1, 0, 3)
    return np.ascontiguousarray(a).astype(NP_BF16).reshape(N // P, P, K)


def kernel(x_t, h_tm1, input_weight, hidden_state_weight, bias):
    x_t = np.asarray(x_t, dtype=np.float32)
    h_tm1 = np.asarray(h_tm1, dtype=np.float32)
    input_weight = np.asarray(input_weight, dtype=np.float32)
    hidden_state_weight = np.asarray(hidden_state_weight, dtype=np.float32)
    bias = np.asarray(bias, dtype=np.float32)

    u = UNITS
    # Gate weights: [x; h] @ [Wi[:, :2u]; Wh[:, :2u]]
    w_gate = np.concatenate(
        [input_weight[:, : 2 * u], hidden_state_weight[:, : 2 * u]], axis=0
    )  # [4096, 4096]
    w_cand = np.concatenate(
        [input_weight[:, 2 * u:], hidden_state_weight[:, 2 * u:]], axis=0
    )  # [4096, 2048]
    def _pair(w):  # [T, 128, K] -> [T/2, 128, 2K]: pairs contiguous per DMA
        T, p, K = w.shape
        return np.ascontiguousarray(
            w.reshape(T // 2, 2, p, K).transpose(0, 2, 1, 3)
        ).reshape(T // 2, p, 2 * K)

    w_g_np = _pair(_pack_w(w_gate))  # [16, 128, 8192] bf16
    w_c_np = _pair(_pack_w(w_cand))  # [8, 128, 8192] bf16
    b_g_np = np.ascontiguousarray(bias[: 2 * u].reshape(NT_G, P).T, dtype=np.float32)
    b_c_np = np.ascontiguousarray(bias[2 * u:].reshape(NT_C, P).T, dtype=np.float32)

    in_maps = []
    for i in range(N_CORES):
        sl = slice(i * B_LOC, (i + 1) * B_LOC)
        xT_np = x_t[sl].T.astype(NP_BF16).reshape(KT_X, P, B_LOC)
        hT_np = h_tm1[sl].T.astype(NP_BF16).reshape(KT_H, P, B_LOC)
        in_maps.append(
            {
                "xT": np.ascontiguousarray(xT_np),
                "hT": np.ascontiguousarray(hT_np),
                "w_g": w_g_np,
                "w_c": w_c_np,
                "b_g": b_g_np,
                "b_c": b_c_np,
            }
        )

    nc = _get_nc()
    res = run_bass_kernel_spmd(
        nc, in_maps, core_ids=list(range(N_CORES)), trace=TRACE
    )
    global LAST_RESULTS
    LAST_RESULTS = res

    h_t = np.empty((BATCH, UNITS), dtype=np.float32)
    for i in range(N_CORES):
        o = np.asarray(res.results[i]["out"], dtype=np.float32)
        h_t[i * B_LOC:(i + 1) * B_LOC] = o.reshape(UNITS, B_LOC).T
    return h_t



# revision 17
# speedup vs baseline: 1.2943x; 1.2943x over previous
"""GRU cell (AnotherGRUCell) on 8 TRN2 NeuronCores.

Strategy: pure data-parallel over batch (8192 rows -> 1024 rows/core),
weights replicated. No collectives.

All on-chip compute is in TRANSPOSED layout (units on the partition
axis, batch on the free axis), as in the bf16 baseline, but with a
mixed fp8/fp16 precision scheme chosen from a host-side error model
(validated against HW to 4 digits on the bf16 version):

  - matmul halves run either as fp8e4 (e4m3) DoubleRow matmuls (2
    contraction rows per PE cell per cycle -> ~2x bf16 throughput) or
    as fp16 matmuls (same speed as bf16 but 10-bit mantissa, which
    removes nearly all of the bf16 quantization error).
  - error budget (rel tol 2e-2): per-half err^2 contributions measured
    on the real inputs: r-gate halves ~0.15e-4 each, u-gate halves
    ~1.63e-4, cand-x 4.04e-4, cand-rh 1.29e-4. Chosen config: r fully
    fp8, cand rh-half fp8, u x-half partially fp8 (N8_UX pairs),
    everything else fp16 -> sim rel err ~1.5e-2.
  - all weights (both dtypes) are pre-scaled by S=32 so fp8 weights
    avoid denormals; the sigmoid/tanh activation reads PSUM with
    scale=1/S, folding the rescale into the existing ScalarE op.

Layouts: fp8 moving operands are packed as [128, 2, B] pair tiles
(DoubleRow wants 3D APs [K=128, Ko=2, N] over two adjacent 128-row
k-subtiles); fp8 weights as [128, 2*len8, 128] col-pair slabs; fp16
weights as [128, len16, 128] per-col-tile slabs.
"""

import numpy as np
import ml_dtypes

import concourse.bacc as bacc
import concourse.tile as tile
import concourse.mybir as mybir
from concourse.bass_utils import run_bass_kernel_spmd

N_CORES = 8
UNITS = 2048
IN_DIM = 2048
BATCH = 8192
B_LOC = BATCH // N_CORES  # 1024 batch rows per core

P = 128
KSUB = 32            # 32 contraction k-subtiles for [x; h] (16 x + 16 h)
NT = UNITS // P      # 16 col-tiles per gate (r / u / cand)
M_CHUNK = 512
MC = B_LOC // M_CHUNK  # 2 moving chunks per core

S = 32.0             # weight pre-scale (fp8 denormal avoidance)

# fp8 coverage (in DoubleRow k-pairs, each pair = 2 k-subtiles of 128):
# r: both halves fully fp8; u: N8_UX pairs of the x-half; c: rh-half.
N8_UX = 4
LEN8_U = 2 * N8_UX          # fp8 k-subtiles per u col-tile
LEN16_U = (16 - LEN8_U) + 16  # fp16 k-subtiles per u col-tile

F8 = mybir.dt.float8e4
F16 = mybir.dt.float16
F32 = mybir.dt.float32
BF16 = mybir.dt.bfloat16
NP_F8 = ml_dtypes.float8_e4m3
NP_F16 = np.float16
DR = mybir.MatmulPerfMode.DoubleRow

_CACHED_NC = None

# test.py sets TRACE=True to capture the NTFF profile (exec_time_ns +
# perfetto trace); the graded path leaves it off. LAST_RESULTS holds the
# BassKernelResults of the most recent run.
TRACE = False
LAST_RESULTS = None


def _build():
    nc = bacc.Bacc("TRN2", target_bir_lowering=False, debug=False)

    x8d = nc.dram_tensor("x8d", [8, P, 2, B_LOC], F8, kind="ExternalInput")
    h8d = nc.dram_tensor("h8d", [8, P, 2, B_LOC], F8, kind="ExternalInput")
    # fp16 moving tensors in groups of 4 k-subtiles (fewer tiles -> fewer
    # PE first-use semaphore beats)
    x16d = nc.dram_tensor("x16d", [4, P, 4, B_LOC], F16, kind="ExternalInput")
    h16d = nc.dram_tensor("h16d", [4, P, 4, B_LOC], F16, kind="ExternalInput")
    # weights: fp8 slabs per col-PAIR [pair, 128, 2*len8, 128]; fp16
    # slabs per col-tile [t, 128, len16, 128]
    wr8 = nc.dram_tensor("wr8", [8, P, 2 * KSUB, P], F8, kind="ExternalInput")
    wu8 = nc.dram_tensor("wu8", [8, P, 2 * LEN8_U, P], F8, kind="ExternalInput")
    wu16 = nc.dram_tensor("wu16", [16, P, LEN16_U, P], F16, kind="ExternalInput")
    wc16 = nc.dram_tensor("wc16", [16, P, 16, P], F16, kind="ExternalInput")
    wc8 = nc.dram_tensor("wc8", [8, P, 2 * 16, P], F8, kind="ExternalInput")
    brd = nc.dram_tensor("brd", [P, NT], F32, kind="ExternalInput")
    bud = nc.dram_tensor("bud", [P, NT], F32, kind="ExternalInput")
    bcd = nc.dram_tensor("bcd", [P, NT], F32, kind="ExternalInput")
    out = nc.dram_tensor("out", [NT, P, B_LOC], F32, kind="ExternalOutput")

    SIG = mybir.ActivationFunctionType.Sigmoid
    TANH = mybir.ActivationFunctionType.Tanh
    INV_S = 1.0 / S

    with tile.TileContext(nc) as tc:
        with (
            tc.tile_pool(name="resident", bufs=1) as res,
            tc.tile_pool(name="wslab", bufs=2) as wp,
            tc.tile_pool(name="psum", bufs=8, space="PSUM") as pp,
            tc.tile_pool(name="stage", bufs=2) as sp,
            tc.tile_pool(name="bias", bufs=1) as bp,
        ):
            x8t = [res.tile([P, 2, B_LOC], F8, tag=f"x8{j}", name=f"x8{j}")
                   for j in range(8)]
            h8t = [res.tile([P, 2, B_LOC], F8, tag=f"h8{j}", name=f"h8{j}")
                   for j in range(8)]
            x16g = [res.tile([P, 4, B_LOC], F16, tag=f"x16g{g}",
                             name=f"x16g{g}") for g in range(4)]
            h16g = [res.tile([P, 4, B_LOC], F16, tag=f"h16g{g}",
                             name=f"h16g{g}") for g in range(4)]
            rh8t = [res.tile([P, 2, B_LOC], F8, tag=f"rh{j}", name=f"rh{j}")
                    for j in range(8)]
            # u16[0..7] are fresh buffers; u16[8..15] alias the h8 pair
            # tiles (same 2KB/partition footprint), which go dead once the
            # r phase ends -- the tile framework sequences the reuse.
            u16t = [res.tile([P, B_LOC], F16, tag=f"u{j}", name=f"u{j}")
                    for j in range(8)] + [None] * 8

            def x16_ap(j, ms):
                return x16g[j // 4][:, j % 4, ms]

            def h16_ap(j, ms):
                return h16g[j // 4][:, j % 4, ms]

            # PE warm-up vs the HAM clock gate (see bf16 baseline notes):
            # 8 dummy matmuls sized to end before real operands land.
            warm_src = sp.tile([P, M_CHUNK], BF16, tag="warm", name="warm_src",
                               bufs=1)
            nc.gpsimd.memset(warm_src[:], 0.0)
            warm_ps = pp.tile([P, M_CHUNK], F32, tag="psum", name="warm_ps")
            for w in range(8):
                nc.tensor.matmul(
                    warm_ps[:], warm_src[:, :P], warm_src[:],
                    start=(w == 0), stop=(w == 7),
                )

            # biases are tiny; land them first
            br = bp.tile([P, NT], F32, tag="br", name="br")
            nc.sync.dma_start(br[:], brd[:, :])
            bu = bp.tile([P, NT], F32, tag="bu", name="bu")
            nc.scalar.dma_start(bu[:], bud[:, :])
            bc = bp.tile([P, NT], F32, tag="bc", name="bc")
            nc.scalar.dma_start(bc[:], bcd[:, :])

            # ---- startup: first TWO r col-pairs (t=0..3) block-interleaved
            # over 8 PSUM banks: each freshly-landed operand pair feeds 8
            # matmuls (~1.7us of PE work per 384KB of DMA), so the PE
            # streams continuously while the input set loads. x8[0] rides
            # the sync HWDGE ring, split into two slot descriptors, issued
            # first: SWDGE (gpsimd) DMA completion is NOT properly waited
            # on by PE consumers (observed first-run race -> NaN).
            nc.sync.dma_start(x8t[0][:, 0:1, :], x8d[0, :, 0:1, :])
            nc.sync.dma_start(x8t[0][:, 1:2, :], x8d[0, :, 1:2, :])

            CH = [1, 3, 4, 4, 4]
            CB = [0, 1, 4, 8, 12, 16]
            wrf = [[[None] * len(CH) for _ in range(2)] for _ in range(2)]
            rr = 0  # round-robin ring selector
            for c, cw in enumerate(CH):
                for tp in range(2):
                    for ti in range(2):
                        t8 = wp.tile([P, 2 * cw, P], F8,
                                     tag=f"wrf{tp}{ti}_{c}",
                                     name=f"wrf{tp}{ti}_{c}", bufs=1)
                        (nc.sync if rr % 2 == 0 else nc.scalar).dma_start(
                            t8[:], wr8[tp, :, ti * KSUB + 2 * CB[c]:
                                        ti * KSUB + 2 * CB[c + 1], :]
                        )
                        rr += 1
                        wrf[tp][ti][c] = t8
                for jp in range(CB[c], CB[c + 1]):
                    if jp == 0:
                        continue  # x8[0] already issued above
                    dst = x8t[jp] if jp < 8 else h8t[jp - 8]
                    src = x8d[jp, :, :, :] if jp < 8 else h8d[jp - 8, :, :, :]
                    (nc.sync if jp % 2 == 0 else nc.scalar).dma_start(
                        dst[:], src)

            def r_src(jp):
                return x8t[jp] if jp < 8 else h8t[jp - 8]

            def act_r(t, m, ps):
                """rh[t] = sigmoid(psum/S + b) * h16[t], stored fp8."""
                ms = slice(m * M_CHUNK, (m + 1) * M_CHUNK)
                rt = sp.tile([P, M_CHUNK], F16, tag="rtmp", name=f"r{t}_{m}")
                nc.scalar.activation(rt[:], ps[:], SIG, bias=br[:, t:t + 1],
                                     scale=INV_S)
                nc.vector.tensor_mul(
                    rh8t[t // 2][:, t % 2, ms], rt[:], h16_ap(t, ms)
                )

            t0_groups = [(t, m) for t in range(4) for m in range(MC)]
            pss0 = [pp.tile([P, M_CHUNK], F32, tag="psum", name=f"psg0_{i}")
                    for i in range(8)]
            for c in range(len(CH)):
                for i, (t, m) in enumerate(t0_groups):
                    ms = slice(m * M_CHUNK, (m + 1) * M_CHUNK)
                    for jp in range(CB[c], CB[c + 1]):
                        wch = wrf[t // 2][t % 2][c]
                        jj = jp - CB[c]
                        nc.tensor.matmul(
                            pss0[i][:],
                            wch[:, 2 * jj:2 * jj + 2, :],
                            r_src(jp)[:, :, ms],
                            start=(jp == 0), stop=(jp == 15),
                            perf_mode=DR,
                        )

            # h16 group 0 feeds the startup epilogues (t=0..3); group 1 the
            # next col-pair's. Triggers execute in order per engine, so
            # placing these right after the startup stream keeps them from
            # stealing startup bandwidth.
            nc.sync.dma_start(h16g[0][:], h16d[0, :, :, :])
            nc.scalar.dma_start(h16g[1][:], h16d[1, :, :, :])

            for i, (t, m) in enumerate(t0_groups):
                act_r(t, m, pss0[i])

            # ---- r steady state: col-pairs 2..7, fully fp8 DoubleRow ----
            def gemm_fp8(ws, base, src_fn, npairs, psl, first, last):
                """m-interleaved DoubleRow accumulation over npairs pairs."""
                for jp in range(npairs):
                    for m in range(MC):
                        ms = slice(m * M_CHUNK, (m + 1) * M_CHUNK)
                        nc.tensor.matmul(
                            psl[m][:],
                            ws[:, base + 2 * jp:base + 2 * jp + 2, :],
                            src_fn(jp)[:, :, ms],
                            start=(first and jp == 0),
                            stop=(last and jp == npairs - 1),
                            perf_mode=DR,
                        )

            for tp in range(2, 8):
                ws = wp.tile([P, 2 * KSUB, P], F8, tag="wr8", name=f"wr8_{tp}",
                             bufs=2)
                (nc.sync if tp % 2 == 0 else nc.scalar).dma_start(
                    ws[:], wr8[tp, :, :, :])
                # remaining bulk fp16 groups, paced behind the slab stream
                if tp == 2:
                    nc.sync.dma_start(h16g[2][:], h16d[2, :, :, :])
                elif tp == 3:
                    nc.scalar.dma_start(h16g[3][:], h16d[3, :, :, :])
                elif tp == 4:
                    nc.sync.dma_start(x16g[2][:], x16d[2, :, :, :])
                elif tp == 5:
                    nc.scalar.dma_start(x16g[3][:], x16d[3, :, :, :])
                elif tp == 6:
                    nc.sync.dma_start(x16g[0][:], x16d[0, :, :, :])
                elif tp == 7:
                    nc.scalar.dma_start(x16g[1][:], x16d[1, :, :, :])
                for ti in range(2):
                    t = 2 * tp + ti
                    psl = [pp.tile([P, M_CHUNK], F32, tag="psum",
                                   name=f"psr{t}_{m}") for m in range(MC)]
                    gemm_fp8(ws, ti * KSUB, r_src, 16, psl, True, True)
                    for m in range(MC):
                        act_r(t, m, psl[m])

            def gemm_fp16(ws, wbase, src_ap, nsub, psl, first, last):
                for j in range(nsub):
                    for m in range(MC):
                        ms = slice(m * M_CHUNK, (m + 1) * M_CHUNK)
                        nc.tensor.matmul(
                            psl[m][:],
                            ws[:, wbase + j:wbase + j + 1, :],
                            src_ap(j, ms),
                            start=(first and j == 0),
                            stop=(last and j == nsub - 1),
                        )

            def act_u(t, m, ps):
                ms = slice(m * M_CHUNK, (m + 1) * M_CHUNK)
                nc.scalar.activation(u16t[t][:, ms], ps[:], SIG,
                                     bias=bu[:, t:t + 1], scale=INV_S)

            wu8_cur = None
            for t in range(NT):
                if t % 2 == 0 and LEN8_U > 0:
                    wu8_cur = wp.tile([P, 2 * LEN8_U, P], F8, tag="wu8",
                                      name=f"wu8_{t // 2}", bufs=2)
                    nc.sync.dma_start(wu8_cur[:], wu8[t // 2, :, :, :])
                w16 = wp.tile([P, LEN16_U, P], F16, tag="wu16",
                              name=f"wu16_{t}", bufs=2)
                nc.scalar.dma_start(w16[:], wu16[t, :, :, :])
                if t >= 8:
                    u16t[t] = res.tile([P, B_LOC], F16, tag=f"h8{t - 8}",
                                       name=f"u{t}")
                psl = [pp.tile([P, M_CHUNK], F32, tag="psum",
                               name=f"psu{t}_{m}") for m in range(MC)]
                if LEN8_U > 0:
                    gemm_fp8(wu8_cur, (t % 2) * LEN8_U,
                             lambda jp: x8t[jp], N8_UX, psl, True, False)
                # fp16 remainder: x-rest subtiles then the full h half
                gemm_fp16(w16, 0, lambda j, ms: x16_ap(LEN8_U + j, ms),
                          16 - LEN8_U, psl, LEN8_U == 0, False)
                gemm_fp16(w16, 16 - LEN8_U, h16_ap, 16, psl, False, True)
                for m in range(MC):
                    act_u(t, m, psl[m])

            # ---- candidate: fp16 x-half + fp8 DoubleRow rh-half ----
            def cand_epilogue(t, m, mw, ps):
                ms = slice(m * mw, (m + 1) * mw)
                cand = sp.tile([P, mw], F32, tag="cand", name=f"c{t}_{m}")
                nc.scalar.activation(cand[:], ps[:], TANH,
                                     bias=bc[:, t:t + 1], scale=INV_S)
                d = sp.tile([P, mw], F32, tag="d", name=f"d{t}_{m}")
                nc.vector.tensor_sub(d[:], h16_ap(t, ms), cand[:])
                d2 = sp.tile([P, mw], F32, tag="d2", name=f"d2{t}_{m}")
                nc.vector.tensor_mul(d2[:], u16t[t][:, ms], d[:])
                nc.vector.tensor_add(d[:], d2[:], cand[:])
                nc.sync.dma_start(out[t, :, ms], d[:])

            wc8_cur = None
            for t in range(NT):
                w16 = wp.tile([P, 16, P], F16, tag="wc16", name=f"wc16_{t}",
                              bufs=2)
                nc.scalar.dma_start(w16[:], wc16[t, :, :, :])
                if t % 2 == 0:
                    wc8_cur = wp.tile([P, 2 * 16, P], F8, tag="wc8",
                                      name=f"wc8_{t // 2}", bufs=2)
                    nc.sync.dma_start(wc8_cur[:], wc8[t // 2, :, :, :])
                if t < NT - 1:
                    psl = [pp.tile([P, M_CHUNK], F32, tag="psum",
                                   name=f"psc{t}_{m}") for m in range(MC)]
                    gemm_fp16(w16, 0, x16_ap, 16, psl, True, False)
                    gemm_fp8(wc8_cur, (t % 2) * 16,
                             lambda jp: rh8t[jp], 8, psl, False, True)
                    for m in range(MC):
                        cand_epilogue(t, m, M_CHUNK, psl[m])
                else:
                    # taper the last col-tile: narrow sequential chunks so
                    # the post-final-matmul tail stays short
                    mw = M_CHUNK // 2
                    for m in range(B_LOC // mw):
                        ms = slice(m * mw, (m + 1) * mw)
                        ps = pp.tile([P, mw], F32, tag="psum",
                                     name=f"psct_{m}")
                        for j in range(16):
                            nc.tensor.matmul(
                                ps[:], w16[:, j:j + 1, :], x16_ap(j, ms),
                                start=(j == 0), stop=False,
                            )
                        for jp in range(8):
                            nc.tensor.matmul(
                                ps[:],
                                wc8_cur[:, 16 + 2 * jp:16 + 2 * jp + 2, :],
                                rh8t[jp][:, :, ms],
                                start=False, stop=(jp == 7),
                                perf_mode=DR,
                            )
                        cand_epilogue(t, m, mw, ps)

    nc.compile()
    return nc


def _get_nc():
    global _CACHED_NC
    if _CACHED_NC is None:
        _CACHED_NC = _build()
    return _CACHED_NC


def _pack_w8(w, subtiles):
    """[4096, 2048] f32 -> [8, 128, 2*len8, 128] e4m3 col-pair slabs.

    slab[tp, p, ti*len8 + i, c] = S * w[subtiles[i]*128 + p, (2tp+ti)*128+c]
    """
    A = (w * S).reshape(KSUB, P, NT, P)[list(subtiles)]  # [len8, p, t, c]
    A = A.transpose(2, 1, 0, 3)  # [t, p, len8, c]
    n8 = len(subtiles)
    A = A.reshape(8, 2, P, n8, P).transpose(0, 2, 1, 3, 4)
    return np.ascontiguousarray(A.reshape(8, P, 2 * n8, P)).astype(NP_F8)


def _pack_w16(w, subtiles):
    """[4096, 2048] f32 -> [16, 128, len16, 128] fp16 per-col-tile slabs."""
    A = (w * S).reshape(KSUB, P, NT, P)[list(subtiles)]
    A = A.transpose(2, 1, 0, 3)  # [t, p, len16, c]
    return np.ascontiguousarray(A).astype(NP_F16)


def _pack_mov8(xT):
    """[2048, 1024] -> [8, 128, 2, 1024] e4m3 DoubleRow pair tiles."""
    A = xT.reshape(8, 2, P, B_LOC).transpose(0, 2, 1, 3)
    return np.ascontiguousarray(A).astype(NP_F8)


def kernel(x_t, h_tm1, input_weight, hidden_state_weight, bias):
    x_t = np.asarray(x_t, dtype=np.float32)
    h_tm1 = np.asarray(h_tm1, dtype=np.float32)
    input_weight = np.asarray(input_weight, dtype=np.float32)
    hidden_state_weight = np.asarray(hidden_state_weight, dtype=np.float32)
    bias = np.asarray(bias, dtype=np.float32)

    u = UNITS
    # per-gate stacked weights [x; h] -> [4096, 2048] each
    w_r = np.concatenate([input_weight[:, :u], hidden_state_weight[:, :u]], 0)
    w_u = np.concatenate(
        [input_weight[:, u:2 * u], hidden_state_weight[:, u:2 * u]], 0)
    w_c = np.concatenate(
        [input_weight[:, 2 * u:], hidden_state_weight[:, 2 * u:]], 0)

    wr8_np = _pack_w8(w_r, range(32))
    wu8_np = _pack_w8(w_u, range(LEN8_U))
    wu16_np = _pack_w16(w_u, list(range(LEN8_U, 16)) + list(range(16, 32)))
    wc16_np = _pack_w16(w_c, range(16))
    wc8_np = _pack_w8(w_c, range(16, 32))
    br_np = np.ascontiguousarray(bias[:u].reshape(NT, P).T, dtype=np.float32)
    bu_np = np.ascontiguousarray(bias[u:2 * u].reshape(NT, P).T,
                                 dtype=np.float32)
    bc_np = np.ascontiguousarray(bias[2 * u:].reshape(NT, P).T,
                                 dtype=np.float32)

    in_maps = []
    for i in range(N_CORES):
        sl = slice(i * B_LOC, (i + 1) * B_LOC)
        xT = np.ascontiguousarray(x_t[sl].T)   # [2048, 1024]
        hT = np.ascontiguousarray(h_tm1[sl].T)
        in_maps.append({
            "x8d": _pack_mov8(xT),
            "h8d": _pack_mov8(hT),
            "x16d": np.ascontiguousarray(
                xT.reshape(4, 4, P, B_LOC).transpose(0, 2, 1, 3)
            ).astype(NP_F16),
            "h16d": np.ascontiguousarray(
                hT.reshape(4, 4, P, B_LOC).transpose(0, 2, 1, 3)
            ).astype(NP_F16),
            "wr8": wr8_np, "wu8": wu8_np, "wu16": wu16_np,
            "wc16": wc16_np, "wc8": wc8_np,
            "brd": br_np, "bud": bu_np, "bcd": bc_np,
        })

    nc = _get_nc()
    res = run_bass_kernel_spmd(
        nc, in_maps, core_ids=list(range(N_CORES)), trace=TRACE
    )
    global LAST_RESULTS
    LAST_RESULTS = res

    h_t = np.empty((BATCH, UNITS), dtype=np.float32)
    for i in range(N_CORES):
        o = np.asarray(res.results[i]["out"], dtype=np.float32)
        h_t[i * B_LOC:(i + 1) * B_LOC] = o.reshape(UNITS, B_LOC).T
    return h_t


# revision 22
# speedup vs baseline: 1.4651x; 1.1320x over previous
"""GRU cell (AnotherGRUCell) on 8 TRN2 NeuronCores.

Strategy: pure data-parallel over batch (8192 rows -> 1024 rows/core),
weights replicated. No collectives.

All on-chip compute is in TRANSPOSED layout (units on the partition
axis, batch on the free axis), as in the bf16 baseline, but with a
mixed fp8/fp16 precision scheme chosen from a host-side error model
(validated against HW to 4 digits on the bf16 version):

  - matmul halves run either as fp8e4 (e4m3) DoubleRow matmuls (2
    contraction rows per PE cell per cycle -> ~2x bf16 throughput) or
    as fp16 matmuls (same speed as bf16 but 10-bit mantissa, which
    removes nearly all of the bf16 quantization error).
  - error budget (rel tol 2e-2): per-half err^2 contributions measured
    on the real inputs: r-gate halves ~0.15e-4 each, u-gate halves
    ~1.63e-4, cand-x 4.04e-4, cand-rh 1.29e-4. Chosen config: r fully
    fp8, cand rh-half fp8, u x-half partially fp8 (N8_UX pairs),
    everything else fp16 -> sim rel err ~1.5e-2.
  - all weights (both dtypes) are pre-scaled by S=32 so fp8 weights
    avoid denormals; the sigmoid/tanh activation reads PSUM with
    scale=1/S, folding the rescale into the existing ScalarE op.

Layouts: fp8 moving operands are packed as [128, 2, B] pair tiles
(DoubleRow wants 3D APs [K=128, Ko=2, N] over two adjacent 128-row
k-subtiles); fp8 weights as [128, 2*len8, 128] col-pair slabs; fp16
weights as [128, len16, 128] per-col-tile slabs.
"""

import numpy as np
import ml_dtypes

import concourse.bacc as bacc
import concourse.tile as tile
import concourse.mybir as mybir
from concourse.bass_utils import run_bass_kernel_spmd

N_CORES = 8
UNITS = 2048
IN_DIM = 2048
BATCH = 8192
B_LOC = BATCH // N_CORES  # 1024 batch rows per core

P = 128
KSUB = 32            # 32 contraction k-subtiles for [x; h] (16 x + 16 h)
NT = UNITS // P      # 16 col-tiles per gate (r / u / cand)
M_CHUNK = 512
MC = B_LOC // M_CHUNK  # 2 moving chunks per core

S = 32.0             # weight pre-scale (fp8 denormal avoidance)

# fp8 coverage (in DoubleRow k-pairs, each pair = 2 k-subtiles of 128):
# r: both halves fully fp8; u: N8_UX pairs of the x-half; c: rh-half.
N8_UX = 8
LEN8_U = 2 * N8_UX          # fp8 k-subtiles per u col-tile
LEN16_U = (16 - LEN8_U) + 16  # fp16 k-subtiles per u col-tile

F8 = mybir.dt.float8e4
F16 = mybir.dt.float16
F32 = mybir.dt.float32
BF16 = mybir.dt.bfloat16
NP_F8 = ml_dtypes.float8_e4m3
NP_F16 = np.float16
DR = mybir.MatmulPerfMode.DoubleRow

_CACHED_NC = None

# test.py sets TRACE=True to capture the NTFF profile (exec_time_ns +
# perfetto trace); the graded path leaves it off. LAST_RESULTS holds the
# BassKernelResults of the most recent run.
TRACE = False
LAST_RESULTS = None


def _build():
    nc = bacc.Bacc("TRN2", target_bir_lowering=False, debug=False)

    x8d = nc.dram_tensor("x8d", [8, P, 2, B_LOC], F8, kind="ExternalInput")
    h8d = nc.dram_tensor("h8d", [8, P, 2, B_LOC], F8, kind="ExternalInput")
    # fp16 moving tensors in groups of 4 k-subtiles (fewer tiles -> fewer
    # PE first-use semaphore beats)
    x16d = nc.dram_tensor("x16d", [4, P, 4, B_LOC], F16, kind="ExternalInput")
    h16d = nc.dram_tensor("h16d", [4, P, 4, B_LOC], F16, kind="ExternalInput")
    # weights: fp8 slabs per col-PAIR [pair, 128, 2*len8, 128]; fp16
    # slabs per col-tile [t, 128, len16, 128]
    wr8 = nc.dram_tensor("wr8", [8, P, 2 * KSUB, P], F8, kind="ExternalInput")
    wu8 = nc.dram_tensor("wu8", [8, P, 2 * LEN8_U, P], F8, kind="ExternalInput")
    wu16 = nc.dram_tensor("wu16", [16, P, LEN16_U, P], F16, kind="ExternalInput")
    wc16 = nc.dram_tensor("wc16", [16, P, 16, P], F16, kind="ExternalInput")
    wc8 = nc.dram_tensor("wc8", [8, P, 2 * 16, P], F8, kind="ExternalInput")
    brd = nc.dram_tensor("brd", [P, NT], F32, kind="ExternalInput")
    bud = nc.dram_tensor("bud", [P, NT], F32, kind="ExternalInput")
    bcd = nc.dram_tensor("bcd", [P, NT], F32, kind="ExternalInput")
    out = nc.dram_tensor("out", [NT, P, B_LOC], F32, kind="ExternalOutput")

    SIG = mybir.ActivationFunctionType.Sigmoid
    TANH = mybir.ActivationFunctionType.Tanh
    INV_S = 1.0 / S

    with tile.TileContext(nc) as tc:
        with (
            tc.tile_pool(name="resident", bufs=1) as res,
            tc.tile_pool(name="wslab", bufs=2) as wp,
            tc.tile_pool(name="psum", bufs=8, space="PSUM") as pp,
            tc.tile_pool(name="stage", bufs=2) as sp,
            tc.tile_pool(name="bias", bufs=1) as bp,
        ):
            x8t = [res.tile([P, 2, B_LOC], F8, tag=f"x8{j}", name=f"x8{j}")
                   for j in range(8)]
            h8t = [res.tile([P, 2, B_LOC], F8, tag=f"h8{j}", name=f"h8{j}")
                   for j in range(8)]
            x16g = [res.tile([P, 4, B_LOC], F16, tag=f"x16g{g}",
                             name=f"x16g{g}") for g in range(4)]
            h16g = [res.tile([P, 4, B_LOC], F16, tag=f"h16g{g}",
                             name=f"h16g{g}") for g in range(4)]
            rh8t = [res.tile([P, 2, B_LOC], F8, tag=f"rh{j}", name=f"rh{j}")
                    for j in range(8)]
            # u16[0..7] are fresh buffers; u16[8..15] alias the h8 pair
            # tiles (same 2KB/partition footprint), which go dead once the
            # r phase ends -- the tile framework sequences the reuse.
            u16t = [res.tile([P, B_LOC], F16, tag=f"u{j}", name=f"u{j}")
                    for j in range(8)] + [None] * 8

            def x16_ap(j, ms):
                return x16g[j // 4][:, j % 4, ms]

            def h16_ap(j, ms):
                return h16g[j // 4][:, j % 4, ms]

            # PE warm-up vs the HAM clock gate (see bf16 baseline notes):
            # 8 dummy matmuls sized to end before real operands land.
            warm_src = sp.tile([P, M_CHUNK], BF16, tag="warm", name="warm_src",
                               bufs=1)
            nc.gpsimd.memset(warm_src[:], 0.0)
            warm_ps = pp.tile([P, M_CHUNK], F32, tag="psum", name="warm_ps")
            for w in range(8):
                nc.tensor.matmul(
                    warm_ps[:], warm_src[:, :P], warm_src[:],
                    start=(w == 0), stop=(w == 7),
                )

            # biases are tiny; land them first
            br = bp.tile([P, NT], F32, tag="br", name="br")
            nc.sync.dma_start(br[:], brd[:, :])
            bu = bp.tile([P, NT], F32, tag="bu", name="bu")
            nc.scalar.dma_start(bu[:], bud[:, :])
            bc = bp.tile([P, NT], F32, tag="bc", name="bc")
            nc.scalar.dma_start(bc[:], bcd[:, :])

            # ---- startup: first TWO r col-pairs (t=0..3) block-interleaved
            # over 8 PSUM banks: each freshly-landed operand pair feeds 8
            # matmuls (~1.7us of PE work per 384KB of DMA), so the PE
            # streams continuously while the input set loads. x8[0] rides
            # the sync HWDGE ring, split into two slot descriptors, issued
            # first: SWDGE (gpsimd) DMA completion is NOT properly waited
            # on by PE consumers (observed first-run race -> NaN).
            nc.sync.dma_start(x8t[0][:, 0:1, :], x8d[0, :, 0:1, :])
            nc.sync.dma_start(x8t[0][:, 1:2, :], x8d[0, :, 1:2, :])

            # tail-graduated chunks: small bites near the end so any DMA
            # late-arrival stalls the PE in <3.4us pieces (no HAM re-dip)
            CH = [1, 3, 4, 4, 2, 1, 1]
            CB = [0, 1, 4, 8, 12, 14, 15, 16]
            wrf = [[[None] * len(CH) for _ in range(2)] for _ in range(2)]
            ws_pre = {}  # pre-streamed steady slabs for col-pairs 2,3
            rr = 0  # round-robin ring selector
            for c, cw in enumerate(CH):
                for tp in range(2):
                    for ti in range(2):
                        t8 = wp.tile([P, 2 * cw, P], F8,
                                     tag=f"wrf{tp}{ti}_{c}",
                                     name=f"wrf{tp}{ti}_{c}", bufs=1)
                        (nc.sync if rr % 2 == 0 else nc.scalar).dma_start(
                            t8[:], wr8[tp, :, ti * KSUB + 2 * CB[c]:
                                        ti * KSUB + 2 * CB[c + 1], :]
                        )
                        rr += 1
                        wrf[tp][ti][c] = t8
                for jp in range(CB[c], CB[c + 1]):
                    if jp == 0:
                        continue  # x8[0] already issued above
                    dst = x8t[jp] if jp < 8 else h8t[jp - 8]
                    src = x8d[jp, :, :, :] if jp < 8 else h8d[jp - 8, :, :, :]
                    (nc.sync if jp % 2 == 0 else nc.scalar).dma_start(
                        dst[:], src)
                # mid-stream insertions: h16 group 0 feeds the startup
                # epilogues (~32us); col-pairs 2/3's full slabs ride the
                # startup stream so the r steady state never waits on a
                # fresh trigger.
                if c == 2:
                    nc.sync.dma_start(h16g[0][:], h16d[0, :, :, :])
                elif c == 3:
                    for tpp in (2, 3):
                        ws = wp.tile([P, 2 * KSUB, P], F8, tag="wr8",
                                     name=f"wr8_{tpp}", bufs=2)
                        (nc.sync if tpp % 2 == 0 else nc.scalar).dma_start(
                            ws[:], wr8[tpp, :, :, :])
                        ws_pre[tpp] = ws

            def r_src(jp):
                return x8t[jp] if jp < 8 else h8t[jp - 8]

            def act_r(t, m, ps):
                """rh[t] = sigmoid(psum/S + b) * h16[t], stored fp8."""
                ms = slice(m * M_CHUNK, (m + 1) * M_CHUNK)
                rt = sp.tile([P, M_CHUNK], F16, tag="rtmp", name=f"r{t}_{m}")
                nc.scalar.activation(rt[:], ps[:], SIG, bias=br[:, t:t + 1],
                                     scale=INV_S)
                nc.vector.tensor_mul(
                    rh8t[t // 2][:, t % 2, ms], rt[:], h16_ap(t, ms)
                )

            t0_groups = [(t, m) for t in range(4) for m in range(MC)]
            pss0 = [pp.tile([P, M_CHUNK], F32, tag="psum", name=f"psg0_{i}")
                    for i in range(8)]
            for c in range(len(CH)):
                for i, (t, m) in enumerate(t0_groups):
                    ms = slice(m * M_CHUNK, (m + 1) * M_CHUNK)
                    for jp in range(CB[c], CB[c + 1]):
                        wch = wrf[t // 2][t % 2][c]
                        jj = jp - CB[c]
                        nc.tensor.matmul(
                            pss0[i][:],
                            wch[:, 2 * jj:2 * jj + 2, :],
                            r_src(jp)[:, :, ms],
                            start=(jp == 0), stop=(jp == 15),
                            perf_mode=DR,
                        )

            # h16 group 1 feeds col-pair 2/3's epilogues (~50us out)
            nc.scalar.dma_start(h16g[1][:], h16d[1, :, :, :])

            for i, (t, m) in enumerate(t0_groups):
                act_r(t, m, pss0[i])

            # ---- r steady state: col-pairs 2..7, fully fp8 DoubleRow ----
            def gemm_fp8(ws, base, src_fn, npairs, psl, first, last):
                """m-interleaved DoubleRow accumulation over npairs pairs."""
                for jp in range(npairs):
                    for m in range(MC):
                        ms = slice(m * M_CHUNK, (m + 1) * M_CHUNK)
                        nc.tensor.matmul(
                            psl[m][:],
                            ws[:, base + 2 * jp:base + 2 * jp + 2, :],
                            src_fn(jp)[:, :, ms],
                            start=(first and jp == 0),
                            stop=(last and jp == npairs - 1),
                            perf_mode=DR,
                        )

            for tp in range(2, 8):
                if tp in ws_pre:
                    ws = ws_pre[tp]
                else:
                    ws = wp.tile([P, 2 * KSUB, P], F8, tag="wr8",
                                 name=f"wr8_{tp}", bufs=2)
                    (nc.sync if tp % 2 == 0 else nc.scalar).dma_start(
                        ws[:], wr8[tp, :, :, :])
                # remaining bulk fp16 groups, paced behind the slab stream
                if tp == 2:
                    nc.sync.dma_start(h16g[2][:], h16d[2, :, :, :])
                elif tp == 3:
                    nc.scalar.dma_start(h16g[3][:], h16d[3, :, :, :])
                elif tp == 4:
                    nc.sync.dma_start(x16g[2][:], x16d[2, :, :, :])
                elif tp == 5:
                    nc.scalar.dma_start(x16g[3][:], x16d[3, :, :, :])
                elif tp == 6:
                    nc.sync.dma_start(x16g[0][:], x16d[0, :, :, :])
                elif tp == 7:
                    nc.scalar.dma_start(x16g[1][:], x16d[1, :, :, :])
                for ti in range(2):
                    t = 2 * tp + ti
                    psl = [pp.tile([P, M_CHUNK], F32, tag="psum",
                                   name=f"psr{t}_{m}") for m in range(MC)]
                    gemm_fp8(ws, ti * KSUB, r_src, 16, psl, True, True)
                    for m in range(MC):
                        act_r(t, m, psl[m])

            def gemm_fp16(ws, wbase, src_ap, nsub, psl, first, last):
                for j in range(nsub):
                    for m in range(MC):
                        ms = slice(m * M_CHUNK, (m + 1) * M_CHUNK)
                        nc.tensor.matmul(
                            psl[m][:],
                            ws[:, wbase + j:wbase + j + 1, :],
                            src_ap(j, ms),
                            start=(first and j == 0),
                            stop=(last and j == nsub - 1),
                        )

            def act_u(t, m, ps):
                ms = slice(m * M_CHUNK, (m + 1) * M_CHUNK)
                nc.scalar.activation(u16t[t][:, ms], ps[:], SIG,
                                     bias=bu[:, t:t + 1], scale=INV_S)

            wu8_cur = None
            for t in range(NT):
                if t % 2 == 0 and LEN8_U > 0:
                    wu8_cur = wp.tile([P, 2 * LEN8_U, P], F8, tag="wu8",
                                      name=f"wu8_{t // 2}", bufs=2)
                    nc.sync.dma_start(wu8_cur[:], wu8[t // 2, :, :, :])
                w16 = wp.tile([P, LEN16_U, P], F16, tag="wu16",
                              name=f"wu16_{t}", bufs=2)
                nc.scalar.dma_start(w16[:], wu16[t, :, :, :])
                if t >= 8:
                    u16t[t] = res.tile([P, B_LOC], F16, tag=f"h8{t - 8}",
                                       name=f"u{t}")
                psl = [pp.tile([P, M_CHUNK], F32, tag="psum",
                               name=f"psu{t}_{m}") for m in range(MC)]
                if LEN8_U > 0:
                    gemm_fp8(wu8_cur, (t % 2) * LEN8_U,
                             lambda jp: x8t[jp], N8_UX, psl, True, False)
                # fp16 remainder: x-rest subtiles then the full h half
                gemm_fp16(w16, 0, lambda j, ms: x16_ap(LEN8_U + j, ms),
                          16 - LEN8_U, psl, LEN8_U == 0, False)
                gemm_fp16(w16, 16 - LEN8_U, h16_ap, 16, psl, False, True)
                for m in range(MC):
                    act_u(t, m, psl[m])

            # ---- candidate: fp16 x-half + fp8 DoubleRow rh-half ----
            def cand_epilogue(t, m, mw, ps):
                ms = slice(m * mw, (m + 1) * mw)
                cand = sp.tile([P, mw], F32, tag="cand", name=f"c{t}_{m}")
                nc.scalar.activation(cand[:], ps[:], TANH,
                                     bias=bc[:, t:t + 1], scale=INV_S)
                d = sp.tile([P, mw], F32, tag="d", name=f"d{t}_{m}")
                nc.vector.tensor_sub(d[:], h16_ap(t, ms), cand[:])
                d2 = sp.tile([P, mw], F32, tag="d2", name=f"d2{t}_{m}")
                nc.vector.tensor_mul(d2[:], u16t[t][:, ms], d[:])
                nc.vector.tensor_add(d[:], d2[:], cand[:])
                nc.sync.dma_start(out[t, :, ms], d[:])

            wc8_cur = None
            for t in range(NT):
                w16 = wp.tile([P, 16, P], F16, tag="wc16", name=f"wc16_{t}",
                              bufs=2)
                nc.scalar.dma_start(w16[:], wc16[t, :, :, :])
                if t % 2 == 0:
                    wc8_cur = wp.tile([P, 2 * 16, P], F8, tag="wc8",
                                      name=f"wc8_{t // 2}", bufs=2)
                    nc.sync.dma_start(wc8_cur[:], wc8[t // 2, :, :, :])
                if t < NT - 1:
                    psl = [pp.tile([P, M_CHUNK], F32, tag="psum",
                                   name=f"psc{t}_{m}") for m in range(MC)]
                    gemm_fp16(w16, 0, x16_ap, 16, psl, True, False)
                    gemm_fp8(wc8_cur, (t % 2) * 16,
                             lambda jp: rh8t[jp], 8, psl, False, True)
                    for m in range(MC):
                        cand_epilogue(t, m, M_CHUNK, psl[m])
                else:
                    # taper the last col-tile: narrow sequential chunks so
                    # the post-final-matmul tail stays short
                    mw = M_CHUNK // 2
                    for m in range(B_LOC // mw):
                        ms = slice(m * mw, (m + 1) * mw)
                        ps = pp.tile([P, mw], F32, tag="psum",
                                     name=f"psct_{m}")
                        for j in range(16):
                            nc.tensor.matmul(
                                ps[:], w16[:, j:j + 1, :], x16_ap(j, ms),
                                start=(j == 0), stop=False,
                            )
                        for jp in range(8):
                            nc.tensor.matmul(
                                ps[:],
                                wc8_cur[:, 16 + 2 * jp:16 + 2 * jp + 2, :],
                                rh8t[jp][:, :, ms],
                                start=False, stop=(jp == 7),
                                perf_mode=DR,
                            )
                        cand_epilogue(t, m, mw, ps)

    nc.compile()
    return nc


def _get_nc():
    global _CACHED_NC
    if _CACHED_NC is None:
        _CACHED_NC = _build()
    return _CACHED_NC


def _pack_w8(w, subtiles):
    """[4096, 2048] f32 -> [8, 128, 2*len8, 128] e4m3 col-pair slabs.

    slab[tp, p, ti*len8 + i, c] = S * w[subtiles[i]*128 + p, (2tp+ti)*128+c]
    """
    A = (w * S).reshape(KSUB, P, NT, P)[list(subtiles)]  # [len8, p, t, c]
    A = A.transpose(2, 1, 0, 3)  # [t, p, len8, c]
    n8 = len(subtiles)
    A = A.reshape(8, 2, P, n8, P).transpose(0, 2, 1, 3, 4)
    return np.ascontiguousarray(A.reshape(8, P, 2 * n8, P)).astype(NP_F8)


def _pack_w16(w, subtiles):
    """[4096, 2048] f32 -> [16, 128, len16, 128] fp16 per-col-tile slabs."""
    A = (w * S).reshape(KSUB, P, NT, P)[list(subtiles)]
    A = A.transpose(2, 1, 0, 3)  # [t, p, len16, c]
    return np.ascontiguousarray(A).astype(NP_F16)


def _pack_mov8(xT):
    """[2048, 1024] -> [8, 128, 2, 1024] e4m3 DoubleRow pair tiles."""
    A = xT.reshape(8, 2, P, B_LOC).transpose(0, 2, 1, 3)
    return np.ascontiguousarray(A).astype(NP_F8)


def kernel(x_t, h_tm1, input_weight, hidden_state_weight, bias):
    x_t = np.asarray(x_t, dtype=np.float32)
    h_tm1 = np.asarray(h_tm1, dtype=np.float32)
    input_weight = np.asarray(input_weight, dtype=np.float32)
    hidden_state_weight = np.asarray(hidden_state_weight, dtype=np.float32)
    bias = np.asarray(bias, dtype=np.float32)

    u = UNITS
    # per-gate stacked weights [x; h] -> [4096, 2048] each
    w_r = np.concatenate([input_weight[:, :u], hidden_state_weight[:, :u]], 0)
    w_u = np.concatenate(
        [input_weight[:, u:2 * u], hidden_state_weight[:, u:2 * u]], 0)
    w_c = np.concatenate(
        [input_weight[:, 2 * u:], hidden_state_weight[:, 2 * u:]], 0)

    wr8_np = _pack_w8(w_r, range(32))
    wu8_np = _pack_w8(w_u, range(LEN8_U))
    wu16_np = _pack_w16(w_u, list(range(LEN8_U, 16)) + list(range(16, 32)))
    wc16_np = _pack_w16(w_c, range(16))
    wc8_np = _pack_w8(w_c, range(16, 32))
    br_np = np.ascontiguousarray(bias[:u].reshape(NT, P).T, dtype=np.float32)
    bu_np = np.ascontiguousarray(bias[u:2 * u].reshape(NT, P).T,
                                 dtype=np.float32)
    bc_np = np.ascontiguousarray(bias[2 * u:].reshape(NT, P).T,
                                 dtype=np.float32)

    in_maps = []
    for i in range(N_CORES):
        sl = slice(i * B_LOC, (i + 1) * B_LOC)
        xT = np.ascontiguousarray(x_t[sl].T)   # [2048, 1024]
        hT = np.ascontiguousarray(h_tm1[sl].T)
        in_maps.append({
            "x8d": _pack_mov8(xT),
            "h8d": _pack_mov8(hT),
            "x16d": np.ascontiguousarray(
                xT.reshape(4, 4, P, B_LOC).transpose(0, 2, 1, 3)
            ).astype(NP_F16),
            "h16d": np.ascontiguousarray(
                hT.reshape(4, 4, P, B_LOC).transpose(0, 2, 1, 3)
            ).astype(NP_F16),
            "wr8": wr8_np, "wu8": wu8_np, "wu16": wu16_np,
            "wc16": wc16_np, "wc8": wc8_np,
            "brd": br_np, "bud": bu_np, "bcd": bc_np,
        })

    nc = _get_nc()
    res = run_bass_kernel_spmd(
        nc, in_maps, core_ids=list(range(N_CORES)), trace=TRACE
    )
    global LAST_RESULTS
    LAST_RESULTS = res

    h_t = np.empty((BATCH, UNITS), dtype=np.float32)
    for i in range(N_CORES):
        o = np.asarray(res.results[i]["out"], dtype=np.float32)
        h_t[i * B_LOC:(i + 1) * B_LOC] = o.reshape(UNITS, B_LOC).T
    return h_t
